# revision 34
# baseline (speedup 1.0000x reference)
"""Trainium2 Bass kernel for 2-layer multi-head GAT (nn_GAT_82867099009055).

Sharding: edges are sharded by DESTINATION range across the 8 cores, so each
dst node's whole in-neighborhood (softmax denominator + weighted sum) is
core-local. Per layer a per-node message table is built by dense matmuls
(fp32r, full rate); the edge phase gathers table rows by src (dma_gather;
int16 indices force a low/high split at 32768). The per-edge attention-src
scalars ride INSIDE the gathered msg row (cols 512:520 of tab1 / col 128 of
tab2); dst-attention scalars come from a tiny core-local bf16 table (16B
rows). Msg columns are (k,h)-interleaved so the e-broadcast multiply keeps a
packed innermost dim and hits the DVE 4x mode. Segment-sum runs on the PE in
bf16: per 128-edge block a one-hot M[j,v]=(dst_local[j]==v) built by one DVE
tensor_scalar feeds psum[v,:] += M.T @ (e*msg); a second matmul with rhs=e
(L1) or rhs=ones with e folded into M (L2) accumulates softmax denominators.
Softmax max-subtract is dropped (scores bounded, ratio invariant). BatchNorm
batch stats come from ones-vector matmuls all-reduced across cores (~4KB);
layer-1 BN is folded into W2 (scale*W2 + shift@W2 row) so x1 stays raw in
SBUF (never hits DRAM). Layer-2 tables are computed on owned rows and
all-gathered in ONE collective (attn scalar included). Output is the owned
dst slice, concatenated on the host.
"""

import sys

for _p in ("/opt/trn_rl_repo",):
    if _p not in sys.path:
        sys.path.insert(0, _p)

from dataclasses import dataclass, field

import numpy as np

import concourse.bass as bass
import concourse.mybir as mybir
import concourse.tile as tile
from concourse import bacc
from concourse.masks import make_identity

FP32 = mybir.dt.float32
FP32R = mybir.dt.float32r
BF16 = mybir.dt.bfloat16
I16 = mybir.dt.int16
AX = mybir.AluOpType
ACT = mybir.ActivationFunctionType


@dataclass
class Cfg:
    N: int = 50000
    E: int = 1600000
    DIN: int = 128
    DH: int = 64
    H: int = 8
    DOUT: int = 128
    CORES: int = 8
    ALPHA: float = 0.2
    BN_EPS: float = 1e-5
    SPLIT: int = 32768          # int16 index range per gather call
    CHUNK: int = 4              # blocks per matmul chunk
    GBLK: int = 8               # blocks per L1 msg-gather call (1024-desc ring)
    GBLK2: int = 8              # blocks per L2 msg-gather call
    GADT: int = 8               # blocks per adt-gather call
    SLAB: int = 4               # 128-node groups per table-build slab

    @property
    def NLOC(self):
        return self.N // self.CORES

    @property
    def NTILES(self):
        return (self.NLOC + 127) // 128

    @property
    def NLOCP(self):
        return self.NTILES * 128

    @property
    def D1(self):
        return self.H * self.DH

    @property
    def T1W(self):
        return 640                  # tab1 row: 512 bf16 msg + 8 fp32 attn-src + pad

    @property
    def T2W(self):
        return 256                  # tab2 row: 128 msg + a2s + pad (512B exactly)


@dataclass
class Sched:
    nb: list = field(default_factory=list)       # [t][h] -> #128-edge blocks
    run_off: list = field(default_factory=list)  # [t][h] -> block offset
    tile_off: list = field(default_factory=list)
    total_blocks: int = 0


def _wrap16(idx, P=128):
    n = idx.shape[0]
    assert n % 16 == 0
    w = idx.reshape(n // 16, 16).T.astype(np.int16)
    return np.ascontiguousarray(np.tile(w, (P // 16, 1)))


def host_prep(cfg, feat, edges, W_heads, a_heads, gamma_h, beta_h,
              W_out, a_out, gamma_o, beta_o, W_res, b_res):
    c = cfg
    src = edges[0].astype(np.int64)
    dst = edges[1].astype(np.int64)
    core_of = dst // c.NLOC

    per_core = []
    cnts = np.zeros((c.CORES, c.NTILES, 2), np.int64)
    for ci in range(c.CORES):
        m = core_of == ci
        s, d = src[m], dst[m]
        dl = d - ci * c.NLOC
        t = dl // 128
        h = (s >= c.SPLIT).astype(np.int64)
        order = np.lexsort((h, t))
        s, dl, t, h = s[order], dl[order], t[order], h[order]
        per_core.append((s, dl, t, h))
        for ti in range(c.NTILES):
            tm = t == ti
            cnts[ci, ti, 0] = int(np.sum(tm & (h == 0)))
            cnts[ci, ti, 1] = int(np.sum(tm & (h == 1)))

    sch = Sched()
    sch.nb = [[int(max(1, -(-int(cnts[:, ti, hh].max()) // 128)))
               for hh in range(2)] for ti in range(c.NTILES)]
    off = 0
    for ti in range(c.NTILES):
        sch.tile_off.append(off)
        sch.run_off.append([off, off + sch.nb[ti][0]])
        off += sch.nb[ti][0] + sch.nb[ti][1]
    sch.total_blocks = off
    TB = sch.total_blocks
    TS = TB * 128

    iota = np.tile(np.arange(128, dtype=np.float32), (128, 1))

    # layer-1 weights, msg columns (k,h)-interleaved: col k*H+h = W_heads[h][:,k]
    W1cat = np.stack([W_heads[hh] for hh in range(c.H)], axis=-1)  # [DIN, DH, H]
    W1cat = W1cat.reshape(c.DIN, c.D1)
    a1s = np.stack([W_heads[hh].astype(np.float64) @ a_heads[hh][:c.DH].astype(np.float64)
                    for hh in range(c.H)], 1).astype(np.float32)
    a1d = np.stack([W_heads[hh].astype(np.float64) @ a_heads[hh][c.DH:].astype(np.float64)
                    for hh in range(c.H)], 1).astype(np.float32)
    W_all1 = np.zeros((c.DIN, c.D1 + 16), np.float32)
    W_all1[:, :c.D1] = W1cat
    W_all1[:, c.D1:c.D1 + c.H] = a1s
    W_all1[:, c.D1 + 8:c.D1 + 8 + c.H] = a1d

    # layer-2: W_out rows reordered to match the (k,h) x1 layout
    perm = np.arange(c.D1).reshape(c.DH, c.H)          # [k, h] -> new idx
    inv = np.empty(c.D1, np.int64)
    for k in range(c.DH):
        for h in range(c.H):
            inv[k * c.H + h] = h * c.DH + k            # old row index
    W2p = W_out[inv, :].astype(np.float64)
    # attn projections for layer 2 (row-permuted to match x1 ordering)
    a2s = (W2p @ a_out[:c.DOUT].astype(np.float64)).astype(np.float32)
    a2d = (W2p @ a_out[c.DOUT:].astype(np.float64)).astype(np.float32)
    W2T = np.concatenate([W2p.astype(np.float32), a2s[:, None], a2d[:, None]],
                         axis=1)                        # [512, 130]

    # gamma/beta permuted to (k,h)
    g1 = gamma_h.T.reshape(-1)    # [k,h] order: gamma_h[h,k] -> idx k*H+h
    b1 = beta_h.T.reshape(-1)
    g1b1 = np.concatenate([g1, b1]).astype(np.float32)[None, :]
    g2b2 = np.concatenate([gamma_o, beta_o]).astype(np.float32)[None, :]
    featT = np.ascontiguousarray(feat.T.astype(np.float32))

    in_maps = []
    for ci in range(c.CORES):
        s, dl, t, h = per_core[ci]
        src_idx = np.zeros(TS, np.int64)
        dloc = np.full(TS, -1.0, np.float32)
        dloc_core = np.zeros(TS, np.int64)
        for ti in range(c.NTILES):
            for hh in range(2):
                m = (t == ti) & (h == hh)
                n = int(m.sum())
                base = sch.run_off[ti][hh] * 128
                src_idx[base:base + n] = s[m] - (c.SPLIT if hh else 0)
                dloc[base:base + n] = (dl[m] - ti * 128).astype(np.float32)
                dloc_core[base:base + n] = dl[m]
        fown = np.zeros((c.DIN, c.NLOCP), np.float32)
        fown[:, :c.NLOC] = featT[:, ci * c.NLOC:(ci + 1) * c.NLOC]
        in_maps.append({
            "featT": featT,
            "featT_own": fown,
            "W_all1": W_all1,
            "W2T": W2T,
            "Wres": W_res.astype(np.float32),
            "bres_rep": np.ascontiguousarray(np.tile(b_res.astype(np.float32)[None, :], (128, 1))),
            "g1b1": g1b1,
            "g2b2": g2b2,
            "iota_rep": iota,
            "gidx_src": _wrap16(src_idx.astype(np.int16)),
            "gidx_dst": _wrap16(dloc_core.astype(np.int16)),
            "dstloc": np.ascontiguousarray(dloc.reshape(TB, 128).T),
        })
    return in_maps, sch


def build_module(cfg, sch):
    c = cfg
    TB = sch.total_blocks
    D1, DO = c.D1, c.DOUT
    T1W, T2W = c.T1W, c.T2W
    NCH1 = D1 // 128
    H = c.H
    nc = bacc.Bacc("TRN2", target_bir_lowering=False, debug=False,
                   enable_asserts=False, num_devices=c.CORES)

    featT = nc.dram_tensor("featT", [c.DIN, c.N], FP32, kind="ExternalInput")
    featT_own = nc.dram_tensor("featT_own", [c.DIN, c.NLOCP], FP32, kind="ExternalInput")
    W_all1 = nc.dram_tensor("W_all1", [c.DIN, D1 + 16], FP32, kind="ExternalInput")
    W2T = nc.dram_tensor("W2T", [D1, DO + 2], FP32, kind="ExternalInput")
    Wres = nc.dram_tensor("Wres", [c.DIN, DO], FP32, kind="ExternalInput")
    bres_rep = nc.dram_tensor("bres_rep", [128, DO], FP32, kind="ExternalInput")
    g1b1 = nc.dram_tensor("g1b1", [1, 2 * D1], FP32, kind="ExternalInput")
    g2b2 = nc.dram_tensor("g2b2", [1, 2 * DO], FP32, kind="ExternalInput")
    iota_rep = nc.dram_tensor("iota_rep", [128, 128], FP32, kind="ExternalInput")
    gidx_src = nc.dram_tensor("gidx_src", [128, TB * 8], I16, kind="ExternalInput")
    gidx_dst = nc.dram_tensor("gidx_dst", [128, TB * 8], I16, kind="ExternalInput")
    dstloc = nc.dram_tensor("dstloc", [128, TB], FP32, kind="ExternalInput")
    out = nc.dram_tensor("out", [c.NLOCP, DO], FP32, kind="ExternalOutput")

    tab1lo = nc.dram_tensor("tab1lo", [c.SPLIT, T1W], BF16)
    tab1hi = nc.dram_tensor("tab1hi", [c.N - c.SPLIT, T1W], BF16)
    adloc1 = nc.dram_tensor("adloc1", [c.NLOCP, 64], FP32)
    tab2mine = nc.dram_tensor("tab2mine", [c.NLOC, T2W], BF16)
    tab2 = nc.dram_tensor("tab2", [c.N, T2W], BF16)
    adloc2 = nc.dram_tensor("adloc2", [c.NLOCP, 64], FP32)

    NGT = (c.N + 127) // 128
    GROUPS = [(0, c.SPLIT), (c.SPLIT, c.N)]
    rg = [list(range(c.CORES))]

    def r3(ap, k):
        return ap.rearrange("p (b k) -> p b k", k=k)

    def f32r(ap):
        return ap.bitcast(FP32R)

    with tile.TileContext(nc) as tc:
        with (
            tc.tile_pool(name="const", bufs=1) as cpool,
            tc.tile_pool(name="setup", bufs=3) as spool,
            tc.tile_pool(name="stage", bufs=3) as gpool,
            tc.tile_pool(name="work", bufs=3) as wpool,
            tc.tile_pool(name="keep", bufs=1) as kpool,
            tc.tile_pool(name="slab", bufs=2) as slpool,
            tc.tile_pool(name="bn", bufs=1) as bnpool,
            tc.tile_pool(name="pe", bufs=2, space="PSUM") as pe_pool,
            tc.tile_pool(name="pstat", bufs=1, space="PSUM") as pstat_pool,
            tc.tile_pool(name="pmisc", bufs=2, space="PSUM") as pmisc_pool,
            tc.tile_pool(name="dram", bufs=1, space="DRAM") as dpool,
        ):
            # ---------------- constants ----------------
            w1_sb = cpool.tile([c.DIN, D1 + 16], FP32)
            nc.sync.dma_start(w1_sb[:], W_all1[:, :])
            w1b = cpool.tile([c.DIN, D1], BF16)
            nc.vector.tensor_copy(w1b[:], w1_sb[:, 0:D1])
            wresb = cpool.tile([c.DIN, DO], BF16)
            iota_sb = cpool.tile([128, 128], FP32)
            nc.sync.dma_start(iota_sb[:], iota_rep[:, :])
            iota_bf = cpool.tile([128, 128], BF16)
            nc.vector.tensor_copy(iota_bf[:], iota_sb[:])
            w2_sb = cpool.tile([128, NCH1 * (DO + 2)], FP32)
            for ch in range(NCH1):
                nc.sync.dma_start(w2_sb[:, ch * (DO + 2):(ch + 1) * (DO + 2)],
                                  W2T[ch * 128:(ch + 1) * 128, :])
            wres_sb = cpool.tile([c.DIN, DO], FP32)
            nc.sync.dma_start(wres_sb[:], Wres[:, :])
            bres_sb = cpool.tile([128, DO], FP32)
            nc.sync.dma_start(bres_sb[:], bres_rep[:, :])
            g1_sb = cpool.tile([1, 2 * D1], FP32)
            nc.sync.dma_start(g1_sb[:], g1b1[:, :])
            g2_sb = cpool.tile([1, 2 * DO], FP32)
            nc.sync.dma_start(g2_sb[:], g2b2[:, :])
            identF = cpool.tile([128, 128], FP32)
            make_identity(nc, identF[:])
            identB = cpool.tile([128, 128], BF16)
            nc.vector.tensor_copy(identB[:], identF[:])
            ones_col = cpool.tile([128, 1], BF16)
            nc.vector.memset(ones_col[:], 1.0)
            ones_row = cpool.tile([1, 128], FP32)
            nc.vector.memset(ones_row[:], 1.0)
            ones_row_bf = cpool.tile([1, 128], BF16)
            nc.vector.memset(ones_row_bf[:], 1.0)

            # own-node dst-attention table + residual matmul (independent of BN)
            resid = kpool.tile([128, c.NTILES * DO], BF16, tag="resid")
            for t0 in range(0, c.NTILES, c.SLAB):
                nt = min(c.SLAB, c.NTILES - t0)
                lhsT4 = slpool.tile([c.DIN, c.SLAB * 128], FP32, tag="ft")
                nc.sync.dma_start(lhsT4[:, 0:nt * 128],
                                  featT_own[:, t0 * 128:(t0 + nt) * 128])
                lhsT4b = slpool.tile([c.DIN, c.SLAB * 128], BF16, tag="ftb")
                nc.vector.tensor_copy(lhsT4b[:, 0:nt * 128], lhsT4[:, 0:nt * 128])
                if t0 == 0:
                    nc.vector.tensor_copy(wresb[:], wres_sb[:])
                asv4 = slpool.tile([128, c.SLAB * 64], FP32, tag="asv")
                for g in range(nt):
                    t = t0 + g
                    lh = lhsT4[:, g * 128:(g + 1) * 128]
                    lhb = lhsT4b[:, g * 128:(g + 1) * 128]
                    ps2 = pe_pool.tile([128, 16], FP32, tag="pB")
                    nc.tensor.matmul(ps2[:], lh, w1_sb[:, D1:D1 + 16],
                                     start=True, stop=True)
                    nc.vector.tensor_copy(asv4[:, g * 64:g * 64 + 8], ps2[:, 8:16])
                    psR = pmisc_pool.tile([128, 512], FP32, tag="mx")
                    nc.tensor.matmul(psR[:, 0:DO], lhb, wresb[:],
                                     start=True, stop=True, skip_group_check=True)
                    nc.vector.tensor_copy(resid[:, t * DO:(t + 1) * DO], psR[:, 0:DO])
                nc.sync.dma_start(
                    adloc1[t0 * 128:(t0 + nt) * 128, :]
                    .rearrange("(g p) w -> p g w", p=128),
                    asv4[:, 0:nt * 64].rearrange("p (g w) -> p g w", w=64))

            # ---------------- layer-1 tables (all nodes, replicated) --------
            # slab-processed: SLAB 128-node groups per DMA to amortize HWDGE
            TROW = D1 + 16
            NSLAB = (NGT + c.SLAB - 1) // c.SLAB
            for sb_i in range(NSLAB):
                g0 = sb_i * c.SLAB
                ng = min(c.SLAB, NGT - g0)
                n0 = g0 * 128
                ncnt = min(c.N, n0 + ng * 128) - n0
                lhsT4 = slpool.tile([c.DIN, c.SLAB * 128], FP32, tag="ft")
                nc.sync.dma_start(lhsT4[:, :ncnt], featT[:, n0:n0 + ncnt])
                lhsT4b = slpool.tile([c.DIN, c.SLAB * 128], BF16, tag="ftb")
                if sb_i % 2 == 0:
                    nc.vector.tensor_copy(lhsT4b[:, :ncnt], lhsT4[:, :ncnt])
                else:
                    nc.scalar.activation(lhsT4b[:, :ncnt], lhsT4[:, :ncnt], ACT.Copy)
                ttile4 = slpool.tile([128, c.SLAB * TROW], BF16, tag="mbf")
                for g in range(ng):
                    cnt = min(128, ncnt - g * 128)
                    lh = lhsT4[:, g * 128:g * 128 + cnt]
                    lhb = lhsT4b[:, g * 128:g * 128 + cnt]
                    ps1 = pe_pool.tile([128, D1], FP32, tag="pA")
                    ps2 = pe_pool.tile([128, 16], FP32, tag="pB")
                    nc.tensor.matmul(ps1[:cnt, :], lhb, w1b[:],
                                     start=True, stop=True)
                    nc.tensor.matmul(ps2[:cnt, :], lh, w1_sb[:, D1:D1 + 16],
                                     start=True, stop=True)
                    tt = ttile4[:, g * TROW:(g + 1) * TROW]
                    if g % 2 == 0:
                        nc.vector.tensor_copy(tt[:cnt, 0:D1], ps1[:cnt, :])
                    else:
                        nc.scalar.activation(tt[:cnt, 0:D1], ps1[:cnt, :], ACT.Copy)
                    nc.vector.tensor_copy(tt[:cnt, D1:D1 + 16].bitcast(FP32),
                                          ps2[:cnt, 0:8])
                # one write for the whole slab; the slab crossing the lo/hi
                # boundary falls back to per-group writes
                crosses = n0 < c.SPLIT < n0 + ng * 128
                tdst, toff = ((tab1lo, 0) if n0 < c.SPLIT
                              else (tab1hi, c.SPLIT))
                if ncnt == ng * 128 and not crosses:
                    nc.sync.dma_start(
                        tdst[n0 - toff:n0 - toff + ncnt, 0:TROW]
                        .rearrange("(g p) w -> p g w", p=128),
                        ttile4[:, 0:ng * TROW].rearrange("p (g w) -> p g w", w=TROW))
                else:
                    for g in range(ng):
                        cnt = min(128, ncnt - g * 128)
                        gn0 = n0 + g * 128
                        gd, go = ((tab1lo, 0) if gn0 < c.SPLIT
                                  else (tab1hi, c.SPLIT))
                        if cnt > 0:
                            nc.sync.dma_start(
                                gd[gn0 - go:gn0 - go + cnt, 0:TROW],
                                ttile4[:cnt, g * TROW:(g + 1) * TROW])

            # ---------------- edge phase (shared for both layers) ----------
            def edge_phase(tabs, adloc, tw, dh_all, nheads, stats_x, stats_sq,
                           x_store, ad_pre=None, x_finish=None):
                # nheads==1: e is folded into M (bf16), denominators via ones
                dhh = dh_all // nheads
                GB = c.GBLK if nheads > 1 else c.GBLK2
                for t in range(c.NTILES):
                    nbl, nbh = sch.nb[t]
                    nbt = nbl + nbh
                    tb0 = sch.tile_off[t]
                    dl_t = gpool.tile([128, nbt], FP32, tag="dl")
                    nc.sync.dma_start(dl_t[:], dstloc[:, tb0:tb0 + nbt])
                    gis_t = gpool.tile([128, nbt * 8], I16, tag="gis")
                    nc.sync.dma_start(gis_t[:],
                                      gidx_src[:, tb0 * 8:(tb0 + nbt) * 8])
                    if ad_pre is None:
                        gid_t = gpool.tile([128, nbt * 8], I16, tag="gid")
                        nc.sync.dma_start(gid_t[:],
                                          gidx_dst[:, tb0 * 8:(tb0 + nbt) * 8])
                        # dst-attention gathers, compressed to 8 fp32 cols
                        adt8 = gpool.tile([128, nbt * 8], FP32, tag="ad8")
                        for a0 in range(0, nbt, c.GADT):
                            ag = min(c.GADT, nbt - a0)
                            adsc = gpool.tile([128, c.GADT * 64], FP32, tag="adsc")
                            nc.gpsimd.dma_gather(
                                r3(adsc[:, 0:ag * 64], 64), adloc[:, :],
                                gid_t[:, a0 * 8:(a0 + ag) * 8],
                                ag * 128, ag * 128, 64)
                            nc.vector.tensor_copy(
                                r3(adt8[:, a0 * 8:(a0 + ag) * 8], 8),
                                r3(adsc[:, 0:ag * 64], 64)[:, :, 0:8])
                    e_t = wpool.tile([128, nbt * nheads], BF16, tag="e")
                    e_f = wpool.tile([128, nbt * nheads], FP32, tag="ef")
                    psA = pe_pool.tile([128, D1], FP32, tag="pA")
                    psBt = pe_pool.tile([128, 16], FP32, tag="pB")
                    psB = psBt[:, 0:nheads]
                    blk = 0
                    for hh, nb in enumerate((nbl, nbh)):
                        ro = 0 if hh == 0 else nbl
                        b0 = sch.run_off[t][hh]
                        tabv = tabs[hh]
                        for c0 in range(0, nb, GB):
                            cg = min(GB, nb - c0)
                            sl0 = b0 + c0          # block idx in gidx_src
                            tl0 = ro + c0          # block idx within tile
                            mst = gpool.tile([128, GB * tw], BF16, tag="ms")
                            nc.gpsimd.dma_gather(
                                r3(mst[:, 0:cg * tw], tw), tabv,
                                gis_t[:, (b0 - tb0 + c0) * 8:
                                      (b0 - tb0 + c0 + cg) * 8],
                                cg * 128, cg * 128, tw)
                            # scores in fp32 -> e (bf16 for L1, fp32 for L2)
                            ast = (r3(mst[:, 0:cg * tw], tw)
                                   [:, :, dh_all:dh_all + 2 * nheads]
                                   .bitcast(FP32))
                            if ad_pre is None:
                                adc = (r3(adt8[:, tl0 * 8:(tl0 + cg) * 8], 8)
                                       [:, :, 0:nheads])
                            else:
                                adc = (ad_pre[:, tb0 + tl0:tb0 + tl0 + cg]
                                       .rearrange("p (b o) -> p b o", o=1))
                            ev = r3(e_f[:], nheads)[:, tl0:tl0 + cg, :]
                            nc.vector.tensor_tensor(ev, ast, adc, op=AX.add)
                            nc.vector.scalar_tensor_tensor(ev, ev, c.ALPHA, ev,
                                                           op0=AX.mult, op1=AX.max)
                            if nheads == 1:
                                nc.scalar.activation(
                                    e_f[:, tl0:tl0 + cg].rearrange(
                                        "p (b o) -> p b o", o=1), ev, ACT.Exp)
                            else:
                                nc.scalar.activation(
                                    r3(e_t[:], nheads)[:, tl0:tl0 + cg, :],
                                    ev, ACT.Exp)
                            # one-hot M for the whole gather chunk in one op
                            mt = wpool.tile([128, GB * 128], BF16, tag="M")
                            if nheads > 1:
                                mv = r3(mt[:, 0:cg * 128], 128)
                                dsl = (dl_t[:, tl0:tl0 + cg]
                                       .rearrange("p (b o) -> p b o", o=1)
                                       .to_broadcast([128, cg, 128]))
                                iot = (iota_sb[:].rearrange("p (o v) -> p o v", o=1)
                                       .to_broadcast([128, cg, 128]))
                                nc.vector.tensor_tensor(mv, dsl, iot,
                                                        op=AX.is_equal)
                            for c1 in range(0, cg, c.CHUNK):
                                cn = min(c.CHUNK, cg - c1)
                                if nheads > 1:
                                    rhs = wpool.tile([128, c.CHUNK * dh_all], BF16,
                                                     tag="rhs")
                                    rv = (r3(rhs[:], dh_all)[:, 0:cn, :]
                                          .rearrange("p b (k h) -> p b k h", h=H))
                                    msrc = (r3(mst[:, 0:cg * tw], tw)
                                            [:, c1:c1 + cn, 0:dh_all]
                                            .rearrange("p b (k h) -> p b k h", h=H))
                                    ein = (r3(e_t[:], nheads)
                                           [:, tl0 + c1:tl0 + c1 + cn, :]
                                           .rearrange("p b (o h) -> p b o h", o=1)
                                           .to_broadcast([128, cn, dhh, H]))
                                    nc.vector.tensor_tensor(rv, msrc, ein,
                                                            op=AX.mult)
                                for j in range(cn):
                                    first, last = blk == 0, blk == nbt - 1
                                    bj = tl0 + c1 + j
                                    mtj = r3(mt[:], 128)[:, c1 + j, :]
                                    if nheads == 1:
                                        nc.vector.tensor_scalar(
                                            mtj, iota_bf[:], dl_t[:, bj:bj + 1],
                                            e_f[:, bj:bj + 1],
                                            op0=AX.is_equal, op1=AX.mult)
                                        nc.tensor.matmul(
                                            psA[:, 0:dh_all], mtj,
                                            r3(mst[:, 0:cg * tw], tw)
                                            [:, c1 + j, 0:dh_all],
                                            start=first, stop=last,
                                            skip_group_check=True)
                                        nc.tensor.matmul(
                                            psB, mtj, ones_col[:],
                                            start=first, stop=last,
                                            skip_group_check=True)
                                    else:
                                        nc.tensor.matmul(
                                            psA[:, 0:dh_all], mtj,
                                            r3(rhs[:], dh_all)[:, j, :],
                                            start=first, stop=last,
                                            skip_group_check=True)
                                        nc.tensor.matmul(
                                            psB, mtj,
                                            r3(e_t[:], nheads)[:, bj, :],
                                            start=first, stop=last,
                                            skip_group_check=True)
                                    blk += 1

                    den = wpool.tile([128, nheads], FP32, tag="den")
                    nc.vector.tensor_scalar_add(den[:], psB, 1e-10)
                    rec = wpool.tile([128, nheads], FP32, tag="rec")
                    nc.vector.reciprocal(rec[:], den[:])
                    xp = x_store(t)
                    if nheads == 1:
                        nc.vector.tensor_scalar(xp, psA[:, 0:dh_all],
                                                rec[:, 0:1], None, op0=AX.mult)
                    else:
                        nc.vector.tensor_tensor(
                            xp.rearrange("p (k h) -> p k h", h=nheads),
                            psA[:, 0:dh_all].rearrange("p (k h) -> p k h", h=nheads),
                            rec[:].rearrange("p (o h) -> p o h", o=1)
                            .to_broadcast([128, dhh, nheads]),
                            op=AX.mult)
                    sq = wpool.tile([128, dh_all], BF16, tag="sq")
                    nc.vector.tensor_tensor(sq[:], xp, xp, op=AX.mult)
                    t0, t1 = t == 0, t == c.NTILES - 1
                    nc.tensor.matmul(stats_x[0:1, 0:dh_all], ones_col[:], xp,
                                     start=t0, stop=t1, skip_group_check=True)
                    nc.tensor.matmul(stats_sq[0:1, 0:dh_all], ones_col[:], sq[:],
                                     start=t0, stop=t1, skip_group_check=True)
                    if x_finish is not None:
                        x_finish(t, xp)

            # ---------------- BN stats -> scale/shift [1, 2*dch] -----------
            def bn_scale_shift(stats_x, stats_sq, g_sb, dch):
                sb = bnpool.tile([1, 2 * dch], FP32, tag="bns")
                nc.vector.tensor_copy(sb[:, 0:dch], stats_x[0:1, 0:dch])
                nc.vector.tensor_copy(sb[:, dch:], stats_sq[0:1, 0:dch])
                bi = dpool.tile([1, 2 * dch], FP32, tag="bnb")
                bo = dpool.tile([1, 2 * dch], FP32, tag="bnb2")
                nc.sync.dma_start(bi[:], sb[:])  # [x-sums | sq-sums]
                nc.gpsimd.collective_compute("AllReduce", AX.add, replica_groups=rg,
                                             ins=[bi.opt()], outs=[bo.opt()])
                gs = bnpool.tile([1, 2 * dch], FP32, tag="bng")
                nc.sync.dma_start(gs[:], bo[:])
                mean = bnpool.tile([1, dch], FP32, tag="bnm")
                nc.vector.tensor_scalar_mul(mean[:], gs[:, 0:dch], 1.0 / c.N)
                tmp = bnpool.tile([1, dch], FP32, tag="bnt")
                nc.vector.tensor_scalar_mul(tmp[:], gs[:, dch:], 1.0 / c.N)
                rs = bnpool.tile([1, dch], FP32, tag="bnrs")
                nc.vector.tensor_tensor(rs[:], mean[:], mean[:], op=AX.mult)
                nc.vector.tensor_tensor(tmp[:], tmp[:], rs[:], op=AX.subtract)
                nc.vector.tensor_scalar_add(tmp[:], tmp[:], c.BN_EPS)
                nc.scalar.activation(tmp[:], tmp[:], ACT.Sqrt)
                nc.vector.reciprocal(rs[:], tmp[:])
                sc = bnpool.tile([1, 2 * dch], FP32, tag="bnsc")
                nc.vector.tensor_tensor(sc[:, 0:dch], g_sb[:, 0:dch], rs[:], op=AX.mult)
                nc.vector.tensor_tensor(sc[:, dch:], mean[:], sc[:, 0:dch], op=AX.mult)
                nc.vector.tensor_tensor(sc[:, dch:], g_sb[:, dch:], sc[:, dch:],
                                        op=AX.subtract)
                return sc

            def replicate(sc, dch, tag):
                rep = kpool.tile([128, 2 * dch], FP32, tag=tag)
                for h0 in range(0, 2 * dch, 512):
                    h1 = min(2 * dch, h0 + 512)
                    psr = pmisc_pool.tile([128, 512], FP32, tag="mx")
                    nc.tensor.matmul(psr[:, 0:h1 - h0], ones_row[:], sc[:, h0:h1],
                                     start=True, stop=True, skip_group_check=True)
                    nc.vector.tensor_copy(rep[:, h0:h1], psr[:, 0:h1 - h0])
                return rep

            # ---------------- layer 1 ----------------
            stats1x = pstat_pool.tile([1, D1], FP32, tag="stx")
            stats1q = pstat_pool.tile([1, D1], FP32, tag="stq")
            x1keep = kpool.tile([128, c.NTILES * D1], BF16, tag="x1")

            def store1(t):
                return x1keep[:, t * D1:(t + 1) * D1]

            def finish1(t, xp):
                # in-place chunk transpose: x1keep tile becomes x1^T blocks
                for ch in range(NCH1):
                    xc = x1keep[:, t * D1 + ch * 128:t * D1 + (ch + 1) * 128]
                    pst = pmisc_pool.tile([128, 512], FP32, tag="mx")
                    pbf = pst[:, 0:256].bitcast(BF16)
                    nc.tensor.transpose(pbf[:, 0:128], xc, identB[:])
                    if ch % 2 == 0:
                        nc.vector.tensor_copy(xc, pbf[:, 0:128])
                    else:
                        nc.scalar.activation(xc, pbf[:, 0:128], ACT.Copy)

            edge_phase([tab1lo[:, :], tab1hi[:, :]], adloc1, T1W, D1, c.H,
                       stats1x[:], stats1q[:], store1, x_finish=finish1)
            sc1 = bn_scale_shift(stats1x[:], stats1q[:], g1_sb, D1)
            rep1 = replicate(sc1, D1, "rep1")

            # fold BN1 into W2: w2s = scale*W2 (bf16), row2 = shift@W2
            s4 = bnpool.tile([128, NCH1], FP32, tag="s4")
            sh4 = bnpool.tile([128, NCH1], FP32, tag="sh4")
            for ch in range(NCH1):
                pst = pmisc_pool.tile([128, 512], FP32, tag="mx")
                nc.tensor.transpose(pst[:, 0:128], rep1[:, ch * 128:(ch + 1) * 128],
                                    identF[:])
                nc.vector.tensor_copy(s4[:, ch:ch + 1], pst[:, 0:1])
                pst2 = pmisc_pool.tile([128, 512], FP32, tag="mx")
                nc.tensor.transpose(pst2[:, 0:128],
                                    rep1[:, D1 + ch * 128:D1 + (ch + 1) * 128],
                                    identF[:])
                nc.vector.tensor_copy(sh4[:, ch:ch + 1], pst2[:, 0:1])
            w2s = cpool.tile([128, NCH1 * (DO + 2)], BF16)
            for ch in range(NCH1):
                nc.vector.tensor_scalar(
                    w2s[:, ch * (DO + 2):(ch + 1) * (DO + 2)],
                    w2_sb[:, ch * (DO + 2):(ch + 1) * (DO + 2)],
                    s4[:, ch:ch + 1], None, op0=AX.mult)
            ps_row = pmisc_pool.tile([128, 512], FP32, tag="mx")
            for ch in range(NCH1):
                nc.tensor.matmul(ps_row[0:1, 0:DO + 2], sh4[:, ch:ch + 1],
                                 w2_sb[:, ch * (DO + 2):(ch + 1) * (DO + 2)],
                                 start=(ch == 0), stop=(ch == NCH1 - 1),
                                 skip_group_check=True)
            row2 = bnpool.tile([1, DO + 2], BF16, tag="row2")
            nc.vector.tensor_copy(row2[:], ps_row[0:1, 0:DO + 2])

            # ---------------- layer 2 prep: tab2 = x1raw @ w2s + row2 ------
            for t0 in range(0, c.NTILES, c.SLAB):
                nt = min(c.SLAB, c.NTILES - t0)
                t24 = slpool.tile([128, c.SLAB * T2W], BF16, tag="m2bf")
                as24 = slpool.tile([128, c.SLAB * 64], FP32, tag="asv")
                for g in range(nt):
                    t = t0 + g
                    xtT = x1keep[:, t * D1:(t + 1) * D1]
                    ps_l2 = pe_pool.tile([128, D1], FP32, tag="pA")
                    for ch in range(NCH1):
                        nc.tensor.matmul(ps_l2[:, 0:DO + 2],
                                         xtT[:, ch * 128:(ch + 1) * 128],
                                         w2s[:, ch * (DO + 2):(ch + 1) * (DO + 2)],
                                         start=(ch == 0), stop=False,
                                         skip_group_check=True)
                    nc.tensor.matmul(ps_l2[:, 0:DO + 2], ones_row_bf[:], row2[:],
                                     start=False, stop=True, skip_group_check=True)
                    nc.vector.tensor_copy(
                        t24[:, g * T2W:g * T2W + DO], ps_l2[:, 0:DO])
                    nc.vector.tensor_copy(
                        t24[:, g * T2W + DO:g * T2W + DO + 2].bitcast(FP32),
                        ps_l2[:, DO:DO + 1])
                    nc.vector.tensor_copy(
                        as24[:, g * 64:g * 64 + 1], ps_l2[:, DO + 1:DO + 2])
                ncnt = min(c.NLOC, (t0 + nt) * 128) - t0 * 128
                if ncnt == nt * 128:
                    nc.sync.dma_start(
                        tab2mine[t0 * 128:t0 * 128 + ncnt, :]
                        .rearrange("(g p) w -> p g w", p=128),
                        t24[:, 0:nt * T2W].rearrange("p (g w) -> p g w", w=T2W))
                else:
                    for g in range(nt):
                        n0 = (t0 + g) * 128
                        cnt = min(128, c.NLOC - n0)
                        if cnt > 0:
                            nc.sync.dma_start(
                                tab2mine[n0:n0 + cnt, :],
                                t24[:cnt, g * T2W:(g + 1) * T2W])
                nc.sync.dma_start(
                    adloc2[t0 * 128:(t0 + nt) * 128, :]
                    .rearrange("(g p) w -> p g w", p=128),
                    as24[:, 0:nt * 64].rearrange("p (g w) -> p g w", w=64))

            # prefetch ALL layer-2 dst-attention values; overlaps the AllGather
            ad2keep = kpool.tile([128, TB], FP32, tag="ad2")
            for g0 in range(0, TB, 64):
                gg = min(64, TB - g0)
                gid64 = gpool.tile([128, 64 * 8], I16, tag="gid64")
                nc.sync.dma_start(gid64[:, 0:gg * 8],
                                  gidx_dst[:, g0 * 8:(g0 + gg) * 8])
                for a1 in range(0, gg, c.GADT):
                    a0 = g0 + a1
                    ag = min(c.GADT, gg - a1)
                    adsc = gpool.tile([128, c.GADT * 64], FP32, tag="adsc")
                    nc.gpsimd.dma_gather(
                        r3(adsc[:, 0:ag * 64], 64), adloc2[:, :],
                        gid64[:, a1 * 8:(a1 + ag) * 8], ag * 128, ag * 128, 64)
                    nc.vector.tensor_copy(
                        ad2keep[:, a0:a0 + ag].rearrange("p (b o) -> p b o", o=1),
                        r3(adsc[:, 0:ag * 64], 64)[:, :, 0:1])

            nc.gpsimd.collective_compute("AllGather", AX.bypass, replica_groups=rg,
                                         ins=[tab2mine[:, :]], outs=[tab2[:, :]])

            # ---------------- layer 2 ----------------
            stats2x = pstat_pool.tile([1, D1], FP32, tag="stx")
            stats2q = pstat_pool.tile([1, D1], FP32, tag="stq")

            def store2(t):
                # x1keep is dead after L2 prep (AllGather barrier); reuse it
                return x1keep[:, t * DO:(t + 1) * DO]

            edge_phase([tab2[0:c.SPLIT, :], tab2[c.SPLIT:c.N, :]], adloc2,
                       T2W, DO, 1, stats2x[:], stats2q[:], store2,
                       ad_pre=ad2keep)
            sc2 = bn_scale_shift(stats2x[:], stats2q[:], g2_sb, DO)
            rep2 = replicate(sc2, DO, "rep2")
            # rep2 shift += bres (saves one op per tile)
            nc.vector.tensor_tensor(rep2[:, DO:], rep2[:, DO:], bres_sb[:],
                                    op=AX.add)

            # ---------------- finalize ----------------
            for t0 in range(0, c.NTILES, c.SLAB):
                nt = min(c.SLAB, c.NTILES - t0)
                o4 = slpool.tile([128, c.SLAB * DO], FP32, tag="o")
                for g in range(nt):
                    t = t0 + g
                    o = o4[:, g * DO:(g + 1) * DO]
                    nc.vector.tensor_tensor(o, store2(t), rep2[:, 0:DO], op=AX.mult)
                    nc.vector.tensor_tensor(o, o, rep2[:, DO:], op=AX.add)
                    nc.vector.tensor_tensor(o, o, resid[:, t * DO:(t + 1) * DO],
                                            op=AX.add)
                nc.sync.dma_start(
                    out[t0 * 128:(t0 + nt) * 128, :]
                    .rearrange("(g p) w -> p g w", p=128),
                    o4[:, 0:nt * DO].rearrange("p (g w) -> p g w", w=DO))

    nc.compile()
    return nc


def kernel(**inputs):
    cfg = Cfg()
    in_maps, sch = host_prep(
        cfg,
        np.asarray(inputs["feat"], np.float32), np.asarray(inputs["edges"]),
        np.asarray(inputs["W_heads"], np.float32), np.asarray(inputs["a_heads"], np.float32),
        np.asarray(inputs["gamma_h"], np.float32), np.asarray(inputs["beta_h"], np.float32),
        np.asarray(inputs["W_out"], np.float32), np.asarray(inputs["a_out"], np.float32),
        np.asarray(inputs["gamma_o"], np.float32), np.asarray(inputs["beta_o"], np.float32),
        np.asarray(inputs["W_res"], np.float32), np.asarray(inputs["b_res"], np.float32))
    nc = build_module(cfg, sch)
    from concourse.bass_utils import run_bass_kernel_spmd
    res = run_bass_kernel_spmd(nc, in_maps, core_ids=list(range(cfg.CORES)))
    outs = [res.results[ci]["out"][:cfg.NLOC] for ci in range(cfg.CORES)]
    return np.concatenate(outs, axis=0).astype(np.float32)


if __name__ == "__main__":
    d = np.load("/root/problem/ref_cache.npz")
    got = kernel(**{k: d[k] for k in d.files if k != "expected"})
    exp = d["expected"]
    err = np.abs(got - exp).max() / np.abs(exp).max()
    print("scale-relative err:", err)
